# revision 5
# baseline (speedup 1.0000x reference)
"""Trainium2 Bass kernel for nn_AttentionBlockV2 (dense transformer block).

Sharding: 8 cores; core c handles batch b=c//4, image row-block r=c%4
(12 of 48 rows = 576 query pixels). Each core computes q/k/v for its FULL
batch (2304 keys; spatially rotated so the program is uniform across cores),
runs attention for its 576 queries over all keys, then the depthwise pos-enc
conv, projection, residuals and the conv-FFN for its local pixels.
No collectives: the host reassembles the 8 local outputs.

Self-contained: hardcodes all shapes; imports only numpy/ml_dtypes/concourse.
"""
import sys
import numpy as np
import ml_dtypes

try:
    import concourse.bass  # noqa: F401
except ImportError:  # fallback when the axon site path isn't preloaded
    sys.path.insert(0, "/opt/trn_rl_repo")

import bass_rust
import concourse.bass as bass
from concourse import bacc
import concourse.mybir as mybir
import concourse.tile as tile
from concourse.vector_clock import ScopedClock
from concourse.bass_utils import run_bass_kernel_spmd

BF16 = ml_dtypes.bfloat16
DT = mybir.dt.bfloat16
F32 = mybir.dt.float32
AF = mybir.ActivationFunctionType
ALU = mybir.AluOpType

# problem constants
B, C, NH, HD, KK, H1 = 2, 256, 8, 32, 7, 512
HS = WS = 48
N = HS * WS            # 2304 keys per batch
NQ = 576               # local queries per core
CH = 288               # query chunk (2 per core)
NKT = N // 128         # 18 key tiles
SCALE = HD ** -0.5
N_CORES = 8

# bias row layout in the packed [16, 128] bias tensor
BIAS_QKV = 0    # rows 0-5: q0,q1,k0,k1,v0,v1 (also v_b for o-norm at rows 4-5)
BIAS_PROJ = 6   # rows 6-7
BIAS_FC1 = 8    # rows 8-11
BIAS_FC2 = 12   # rows 12-13
BIAS_PE = 14    # rows 14-15


def _patched_drain_and_barrier(self, tick_clock, wait_clock):
    # upstream emits one epilogue drain carrying every outstanding wait;
    # walrus codegen accepts at most one sync wait per CTRL instruction,
    # so spread the extras over additional drains.
    drain_inst = self.nc.sync.drain()
    wait_clock.add_sem_waits(drain_inst.ins, ScopedClock({None: tick_clock.global_clock}))
    si = drain_inst.ins.sync_info
    waits = list(si.on_wait) if si is not None else []
    if len(waits) > 1:
        si.on_wait = [waits[0]]
        drain_inst.ins.sync_info = si
        for w in waits[1:]:
            extra = self.nc.sync.drain()
            extra.ins.sync_info = bass_rust.SyncInfo(on_wait=[w], on_update=[])
    self.nc.all_engine_barrier()
    assert self.sems is not None
    popped = self.nc._tile_sem_poison_stack.pop()
    assert popped is self._sem_poison
    self.nc.clear_and_free_semaphores(list(self.sems.allocated().values()))
    self.nc.all_engine_barrier()


tile.TileContext._drain_and_barrier = _patched_drain_and_barrier

# taps executed on the vector engine (rest go to gpsimd); tuned for balance
N_DVE_TAPS = 37


def build_kernel():
    from contextlib import ExitStack

    nc = bacc.Bacc("TRN2", target_bir_lowering=False, debug=False)
    ap_xb = nc.dram_tensor("xb", (2, 128, N), DT, kind="ExternalInput").ap()
    ap_xloc = nc.dram_tensor("xloc", (2, 128, NQ), F32, kind="ExternalInput").ap()
    ap_pemask = nc.dram_tensor("pemask", (128, 18 * 54), DT, kind="ExternalInput").ap()
    ap_wqkv = nc.dram_tensor("wqkv", (2, 128, 768), DT, kind="ExternalInput").ap()
    ap_wvt = nc.dram_tensor("wvt", (2, 128, 256), DT, kind="ExternalInput").ap()
    ap_wproj = nc.dram_tensor("wproj", (2, 128, 256), DT, kind="ExternalInput").ap()
    ap_wfc1 = nc.dram_tensor("wfc1", (2, 128, 512), DT, kind="ExternalInput").ap()
    ap_wfc2 = nc.dram_tensor("wfc2", (4, 128, 256), DT, kind="ExternalInput").ap()
    ap_pew = nc.dram_tensor("pew", (2, 128, 49), F32, kind="ExternalInput").ap()
    ap_bias = nc.dram_tensor("bias", (16, 128), F32, kind="ExternalInput").ap()
    ap_y = nc.dram_tensor("y", (2, 128, NQ), F32, kind="ExternalOutput").ap()

    with tile.TileContext(nc) as tc, ExitStack() as ctx:
        const = ctx.enter_context(tc.tile_pool(name="const", bufs=1))
        persist = ctx.enter_context(tc.tile_pool(name="persist", bufs=1))
        work = ctx.enter_context(tc.tile_pool(name="work", bufs=2))
        ppool = ctx.enter_context(tc.tile_pool(name="pbuf", bufs=2))
        psum = ctx.enter_context(tc.tile_pool(name="psum", bufs=2, space="PSUM"))

        # ---- loads ----
        w_qkv = const.tile([128, 2, 768], DT)
        nc.sync.dma_start(w_qkv[:], ap_wqkv.rearrange("a p m -> p a m"))
        w_vt = const.tile([128, 2, 256], DT)
        nc.sync.dma_start(w_vt[:], ap_wvt.rearrange("a p m -> p a m"))
        w_proj = const.tile([128, 2, 256], DT)
        nc.sync.dma_start(w_proj[:], ap_wproj.rearrange("a p m -> p a m"))
        w_fc1 = const.tile([128, 2, 512], DT)
        nc.sync.dma_start(w_fc1[:], ap_wfc1.rearrange("a p m -> p a m"))
        w_fc2 = const.tile([128, 4, 256], DT)
        nc.sync.dma_start(w_fc2[:], ap_wfc2.rearrange("a p m -> p a m"))
        pew = const.tile([128, 2, 49], F32)
        nc.sync.dma_start(pew[:], ap_pew.rearrange("a p m -> p a m"))
        bias = const.tile([128, 16], F32)
        nc.sync.dma_start(bias[:], ap_bias.rearrange("a p -> p a"))
        pemask = const.tile([128, 18, 54], DT)
        nc.sync.dma_start(pemask[:], ap_pemask.rearrange("p (a b) -> p a b", a=18))
        ones_sb = const.tile([128, 32], DT)
        nc.vector.memset(ones_sb[:], 1.0)

        x_sb = persist.tile([128, 2, N], DT)
        nc.sync.dma_start(x_sb[:], ap_xb.rearrange("a p n -> p a n"))
        xloc = persist.tile([128, 2, NQ], F32)
        nc.sync.dma_start(xloc[:], ap_xloc.rearrange("a p n -> p a n"))

        NCH = [(0, 512), (512, 512), (1024, 512), (1536, 512), (2048, 256)]

        # ---- q conv (local 576 cols only), k conv (full batch) ----
        q_sb = persist.tile([128, 2, NQ], DT)
        for mt in range(2):
            for c0, cw in [(0, 512), (512, 64)]:
                ps = psum.tile([128, 4, 512], F32, tag="ps", name="ps_q")
                for kt in range(2):
                    nc.tensor.matmul(
                        ps[:, 0, :cw],
                        w_qkv[:, kt, 128 * mt:128 * (mt + 1)],
                        x_sb[:, kt, c0:c0 + cw],
                        start=(kt == 0), stop=(kt == 1))
                nc.vector.tensor_scalar_add(q_sb[:, mt, c0:c0 + cw], ps[:, 0, :cw],
                                            bias[:, mt:mt + 1])
        k_sb = persist.tile([128, 2, N], DT)
        for mt in range(2):
            for c0, cw in NCH:
                ps = psum.tile([128, 4, 512], F32, tag="ps", name="ps_k")
                for kt in range(2):
                    nc.tensor.matmul(
                        ps[:, 0, :cw],
                        w_qkv[:, kt, 128 * (2 + mt):128 * (3 + mt)],
                        x_sb[:, kt, c0:c0 + cw],
                        start=(kt == 0), stop=(kt == 1))
                nc.vector.tensor_scalar_add(k_sb[:, mt, c0:c0 + cw], ps[:, 0, :cw],
                                            bias[:, 2 + mt:3 + mt])

        # ---- vT = x^T @ v_w^T : [n, c_v] tiles for the o-matmul lhsT ----
        vt = persist.tile([128, NKT, 256], DT)
        for mt in range(NKT):
            ps = psum.tile([128, 4, 512], F32, tag="ps", name="ps_vt")
            for kt in range(2):
                nc.tensor.matmul(
                    ps[:, 0, :256],
                    x_sb[:, kt, 128 * mt:128 * (mt + 1)],
                    w_vt[:, kt, :],
                    start=(kt == 0), stop=(kt == 1))
            nc.vector.tensor_copy(vt[:, mt, :], ps[:, 0, :256])

        # ---- v conv only over the local window, direct into padded buffer ----
        # window rows (rotated): image rows 45-47 (cols 2160:2304) -> vpad rows 0-2;
        # image rows 0-14 (cols 0:720) -> vpad rows 3-17. borders masked to zero.
        vpad = persist.tile([128, 2, 18, 54], DT)
        nc.gpsimd.memset(vpad[:], 0.0)
        VW = [(2160, 144, 0, 3), (0, 480, 3, 10), (480, 240, 13, 5)]
        for ct in range(2):
            for c0, cw, r0, nr in VW:
                ps = psum.tile([128, 4, 512], F32, tag="ps", name="ps_v")
                for kt in range(2):
                    nc.tensor.matmul(
                        ps[:, 0, :cw],
                        w_qkv[:, kt, 128 * (4 + ct):128 * (5 + ct)],
                        x_sb[:, kt, c0:c0 + cw],
                        start=(kt == 0), stop=(kt == 1))
                nc.vector.scalar_tensor_tensor(
                    vpad[:, ct, r0:r0 + nr, 3:51],
                    ps[:, 0, :cw].rearrange("p (a b) -> p a b", b=48),
                    bias[:, 4 + ct:5 + ct],
                    pemask[:, r0:r0 + nr, 3:51],
                    ALU.add, ALU.mult)

        # shifted copy so odd-dx tap windows are 4B-aligned (DVE 2x mode)
        vpad1 = persist.tile([128, 2, 18, 54], DT)
        nc.vector.tensor_copy(vpad1[:, :, :, 0:53], vpad[:, :, :, 1:54])

        # ---- depthwise 7x7 pos-enc conv over local 12 rows (all on DVE) ----
        pe_a = persist.tile([128, 2, 12, 48], DT)
        pe_b = persist.tile([128, 2, 12, 48], DT)
        taps = [(dy, dx) for dy in range(7) for dx in range(7)]
        for ct in range(2):
            for acc, tap_list in ((pe_a, taps[0::2]), (pe_b, taps[1::2])):
                for i, (dy, dx) in enumerate(tap_list):
                    if dx % 2 == 0:
                        win = vpad[:, ct, dy:dy + 12, dx:dx + 48]
                    else:
                        win = vpad1[:, ct, dy:dy + 12, dx - 1:dx + 47]
                    sc = pew[:, ct, dy * 7 + dx:dy * 7 + dx + 1]
                    if i == 0:
                        b0 = bias[:, BIAS_PE + ct:BIAS_PE + ct + 1] if acc is pe_a else 0.0
                        nc.vector.tensor_scalar(acc[:, ct], win, sc, b0,
                                                ALU.mult, ALU.add)
                    else:
                        nc.vector.scalar_tensor_tensor(acc[:, ct], win, sc, acc[:, ct],
                                                       ALU.mult, ALU.add)

        # ---- attention ----
        proj_in = persist.tile([128, 2, NQ], DT)
        for g in range(2):
            for c in range(2):
                pt = ppool.tile([128, 4, NKT, CH], DT, tag="P")
                for mt in range(NKT):
                    sc = psum.tile([128, 4, 512], F32, tag="ps")
                    for h in range(4):
                        nc.tensor.matmul(
                            sc[:, h, :CH],
                            k_sb[32 * h:32 * h + 32, g, 128 * mt:128 * (mt + 1)],
                            q_sb[32 * h:32 * h + 32, g, CH * c:CH * (c + 1)],
                            start=True, stop=True, tile_position=(32 * h, 0))
                    nc.scalar.activation(pt[:, :, mt, :], sc[:, :, :CH], AF.Exp,
                                         scale=SCALE)
                osum = psum.tile([128, 4, 512], F32, tag="ps")
                for kt in range(NKT):
                    for h in range(4):
                        nc.tensor.matmul(
                            osum[32 * h:32 * h + 32, 0, :CH],
                            vt[:, kt, 128 * g + 32 * h:128 * g + 32 * h + 32],
                            pt[:, h, kt, :],
                            start=(kt == 0), stop=(kt == NKT - 1),
                            tile_position=(0, 32 * h))
                    for h in range(4):
                        nc.tensor.matmul(
                            osum[32 * h:32 * h + 32, 1, :CH],
                            ones_sb[:, 0:32],
                            pt[:, h, kt, :],
                            start=(kt == 0), stop=(kt == NKT - 1),
                            tile_position=(0, 32 * h))
                r_sb = work.tile([128, CH], F32, tag="recip")
                nc.vector.reciprocal(r_sb[:], osum[:, 1, :CH])
                o_tmp = work.tile([128, CH], DT, tag="otmp")
                nc.vector.tensor_tensor(o_tmp[:], osum[:, 0, :CH], r_sb[:], ALU.mult)
                nc.vector.scalar_tensor_tensor(
                    proj_in[:, g, CH * c:CH * (c + 1)], o_tmp[:],
                    bias[:, 4 + g:5 + g], pe_sum_slice(nc, pe_a, pe_b, g, c),
                    ALU.add, ALU.add)

        # ---- proj + residual ----
        x1 = persist.tile([128, 2, NQ], F32)
        x1b = persist.tile([128, 2, NQ], DT)
        for mt in range(2):
            for c in range(2):
                ps = psum.tile([128, 4, 512], F32, tag="ps")
                for kt in range(2):
                    nc.tensor.matmul(
                        ps[:, 0, :CH],
                        w_proj[:, kt, 128 * mt:128 * (mt + 1)],
                        proj_in[:, kt, CH * c:CH * (c + 1)],
                        start=(kt == 0), stop=(kt == 1))
                nc.vector.scalar_tensor_tensor(
                    x1[:, mt, CH * c:CH * (c + 1)], ps[:, 0, :CH],
                    bias[:, BIAS_PROJ + mt:BIAS_PROJ + mt + 1],
                    xloc[:, mt, CH * c:CH * (c + 1)], ALU.add, ALU.add)
                nc.vector.tensor_copy(x1b[:, mt, CH * c:CH * (c + 1)],
                                      x1[:, mt, CH * c:CH * (c + 1)])

        # ---- conv FFN ----
        h_sb = persist.tile([128, 4, NQ], DT)
        for mt in range(4):
            for c in range(2):
                ps = psum.tile([128, 4, 512], F32, tag="ps")
                for kt in range(2):
                    nc.tensor.matmul(
                        ps[:, 0, :CH],
                        w_fc1[:, kt, 128 * mt:128 * (mt + 1)],
                        x1b[:, kt, CH * c:CH * (c + 1)],
                        start=(kt == 0), stop=(kt == 1))
                nc.scalar.activation(h_sb[:, mt, CH * c:CH * (c + 1)], ps[:, 0, :CH],
                                     AF.Silu, bias=bias[:, BIAS_FC1 + mt:BIAS_FC1 + mt + 1])
        y_sb = persist.tile([128, 2, NQ], F32)
        for mt in range(2):
            for c in range(2):
                ps = psum.tile([128, 4, 512], F32, tag="ps")
                for kt in range(4):
                    nc.tensor.matmul(
                        ps[:, 0, :CH],
                        w_fc2[:, kt, 128 * mt:128 * (mt + 1)],
                        h_sb[:, kt, CH * c:CH * (c + 1)],
                        start=(kt == 0), stop=(kt == 3))
                nc.vector.scalar_tensor_tensor(
                    y_sb[:, mt, CH * c:CH * (c + 1)], ps[:, 0, :CH],
                    bias[:, BIAS_FC2 + mt:BIAS_FC2 + mt + 1],
                    x1[:, mt, CH * c:CH * (c + 1)], ALU.add, ALU.add)
            nc.sync.dma_start(ap_y[mt, :, :], y_sb[:, mt, :])
    nc.compile()
    return nc


def pe_sum_slice(nc, pe_a, pe_b, g, c):
    """pe = pe_a + pe_b, materialized lazily per (g, c) slice."""
    # combine the two accumulator chains for this slice into pe_a's buffer
    # is unsafe (pe_a still feeds other slices' reads? no — taps are done),
    # but in-place TT add on distinct tiles is fine; do it once per slice.
    out = pe_a[:, g].rearrange("p a b -> p (a b)")[:, CH * c:CH * (c + 1)]
    bsl = pe_b[:, g].rearrange("p a b -> p (a b)")[:, CH * c:CH * (c + 1)]
    nc.vector.tensor_tensor(out, out, bsl, ALU.add)
    return out


_CACHED = {}


def _get_nc():
    if "nc" not in _CACHED:
        _CACHED["nc"] = build_kernel()
    return _CACHED["nc"]


def _prep_inputs(inputs):
    x = np.asarray(inputs["x"], np.float32)           # [2, 256, 48, 48]
    qk_w = np.asarray(inputs["qk_w"], np.float32)
    qk_b = np.asarray(inputs["qk_b"], np.float32)
    v_w = np.asarray(inputs["v_w"], np.float32)
    v_b = np.asarray(inputs["v_b"], np.float32)
    pe_w = np.asarray(inputs["pe_w"], np.float32)
    pe_b = np.asarray(inputs["pe_b"], np.float32)
    proj_w = np.asarray(inputs["proj_w"], np.float32)
    proj_b = np.asarray(inputs["proj_b"], np.float32)
    fc1_w = np.asarray(inputs["fc1_w"], np.float32)
    fc1_b = np.asarray(inputs["fc1_b"], np.float32)
    fc2_w = np.asarray(inputs["fc2_w"], np.float32)
    fc2_b = np.asarray(inputs["fc2_b"], np.float32)

    rows = np.arange(2 * C).reshape(NH, 2, HD)
    q_rows = rows[:, 0, :].reshape(-1)
    k_rows = rows[:, 1, :].reshape(-1)
    w_qkv = np.concatenate([qk_w[q_rows], qk_w[k_rows], v_w], axis=0)  # [768, 256]
    wqkv = np.ascontiguousarray(w_qkv.T.reshape(2, 128, 768)).astype(BF16)
    wvt = np.ascontiguousarray(v_w.T.reshape(2, 128, 256)).astype(BF16)
    wproj = np.ascontiguousarray(proj_w.T.reshape(2, 128, 256)).astype(BF16)
    wfc1 = np.ascontiguousarray(fc1_w.T.reshape(2, 128, 512)).astype(BF16)
    wfc2 = np.ascontiguousarray(fc2_w.T.reshape(4, 128, 256)).astype(BF16)
    pew = np.ascontiguousarray(pe_w[:, 0].reshape(2, 128, 49)).astype(np.float32)

    bias = np.zeros((16, 128), np.float32)
    bias[0:2] = qk_b[q_rows].reshape(2, 128)
    bias[2:4] = qk_b[k_rows].reshape(2, 128)
    bias[4:6] = v_b.reshape(2, 128)
    bias[6:8] = proj_b.reshape(2, 128)
    bias[8:12] = fc1_b.reshape(4, 128)
    bias[12:14] = fc2_b.reshape(2, 128)
    bias[14:16] = pe_b.reshape(2, 128)

    xn = x.reshape(B, C, HS, WS)
    in_maps = []
    for core in range(N_CORES):
        b, r = core // 4, core % 4
        xrot = np.roll(xn[b], -12 * r, axis=1)                 # rotate rows
        xb = np.ascontiguousarray(xrot.reshape(C, N).reshape(2, 128, N)).astype(BF16)
        xloc = np.ascontiguousarray(
            xrot[:, :12, :].reshape(C, NQ).reshape(2, 128, NQ)).astype(np.float32)
        mask = np.ones((18, 54), np.float32)
        if r == 0:
            mask[0:3, :] = 0.0                                  # top image border
        if r == 3:
            mask[15:18, :] = 0.0                                # bottom image border
        pemask = np.broadcast_to(mask.reshape(1, 972), (128, 972)).astype(BF16)
        in_maps.append({
            "xb": xb, "xloc": xloc, "pemask": np.ascontiguousarray(pemask),
            "wqkv": wqkv, "wvt": wvt, "wproj": wproj, "wfc1": wfc1, "wfc2": wfc2,
            "pew": pew, "bias": bias,
        })
    return in_maps


def kernel(**inputs) -> np.ndarray:
    nc = _get_nc()
    in_maps = _prep_inputs(inputs)
    res = run_bass_kernel_spmd(nc, in_maps, core_ids=list(range(N_CORES)),
                               trace=False)
    out = np.zeros((B, C, HS, WS), np.float32)
    for core in range(N_CORES):
        b, r = core // 4, core % 4
        y = res.results[core]["y"].reshape(C, 12, WS)
        out[b, :, 12 * r:12 * (r + 1), :] = y
    return out


def run_traced(inputs):
    """test-harness helper: run with NTFF tracing, return (out, results)."""
    nc = _get_nc()
    in_maps = _prep_inputs(inputs)
    res = run_bass_kernel_spmd(nc, in_maps, core_ids=list(range(N_CORES)),
                               trace=True)
    out = np.zeros((B, C, HS, WS), np.float32)
    for core in range(N_CORES):
        b, r = core // 4, core % 4
        y = res.results[core]["y"].reshape(C, 12, WS)
        out[b, :, 12 * r:12 * (r + 1), :] = y
    return out, res
